# revision 14
# baseline (speedup 1.0000x reference)
"""Trainium2 Bass kernel for a first-order IIR low-pass filter (v4b).

y_t = alpha * x_t + (1 - alpha) * y_{t-1},  y_{-1} = 0

DVE hardware prefix-scan (TensorTensorScanArith):
    state = (beta * state) + x'_t   (fp32 carried state)
with x' = alpha * x prescaled on the host, shipped bf16, transposed to
[128 sequences = (b, c), T] per core so both DMAs are fully linear.
data0 (the beta multiplier) is a stride-0 broadcast AP unless
IIR_BCAST=0, which materializes a full beta tile instead.
"""

import math
import os
import sys

import numpy as np

try:
    import concourse.bass as bass
except ImportError:
    sys.path.insert(0, "/opt/trn_rl_repo")
    import concourse.bass as bass

import concourse.bacc as bacc
import concourse.mybir as mybir
import concourse.tile as tile
import ml_dtypes
from concourse import bass_utils

SAMPLE_RATE = 16000
CUTOFF_FREQ = 1000.0
_DT = 1.0 / SAMPLE_RATE
_TAU = 1.0 / (2.0 * math.pi * CUTOFF_FREQ)
ALPHA = _DT / (_DT + _TAU)
BETA = 1.0 - ALPHA

B, T, C = 16, 65536, 64
N_CORES = 8
BG = B // N_CORES             # batches per core (2)
P = BG * C                    # partition dim (128 sequences per core)
TC = int(os.environ.get("IIR_TC", "8192"))   # timesteps per chunk
NCH = T // TC

DT_IN = mybir.dt.bfloat16
XBUFS = int(os.environ.get("IIR_XBUFS", "3"))
YBUFS = int(os.environ.get("IIR_YBUFS", "3"))
BCAST = os.environ.get("IIR_BCAST", "1") == "1"

_cached_nc = None


def _build_program():
    nc = bacc.Bacc("TRN2", target_bir_lowering=False, debug=False)

    x_in = nc.dram_tensor("x", [P, T], DT_IN, kind="ExternalInput").ap()
    y_out = nc.dram_tensor("y", [P, T], DT_IN, kind="ExternalOutput").ap()

    mult = mybir.AluOpType.mult
    add = mybir.AluOpType.add

    with tile.TileContext(nc) as tc:
        with (
            tc.tile_pool(name="cst", bufs=1) as cpool,
            tc.tile_pool(name="xin", bufs=XBUFS) as xpool,
            tc.tile_pool(name="ysc", bufs=YBUFS) as ypool,
        ):
            if BCAST:
                beta_col = cpool.tile([P, 1], mybir.dt.float32, tag="beta")
                nc.vector.memset(beta_col[:], BETA)
                beta_ap = beta_col[:].broadcast_to([P, TC])
            else:
                beta_full = cpool.tile([P, TC], mybir.dt.float32, tag="beta")
                nc.vector.memset(beta_full[:], BETA)
                beta_ap = beta_full[:]

            prev = None
            for j in range(NCH):
                xt = xpool.tile([P, TC], DT_IN, tag="xt")
                nc.sync.dma_start(xt[:], x_in[:, j * TC : (j + 1) * TC])

                yt = ypool.tile([P, TC], DT_IN, tag="yt")
                init = 0.0 if prev is None else prev[:, TC - 1 : TC]
                nc.vector.tensor_tensor_scan(
                    yt[:], beta_ap, xt[:], init, mult, add
                )
                prev = yt

                nc.scalar.dma_start(y_out[:, j * TC : (j + 1) * TC], yt[:])

    nc.compile()
    return nc


def _get_program():
    global _cached_nc
    if _cached_nc is None:
        _cached_nc = _build_program()
    return _cached_nc


def _shard_inputs(x):
    xs = (np.float32(ALPHA) * x).astype(ml_dtypes.bfloat16)  # [B, T, C]
    in_maps = []
    for k in range(N_CORES):
        xl = np.ascontiguousarray(
            xs[BG * k : BG * (k + 1)].transpose(0, 2, 1)
        ).reshape(P, T)
        in_maps.append({"x": xl})
    return in_maps


def run(x, trace=False):
    x = np.ascontiguousarray(np.asarray(x, dtype=np.float32))
    assert x.shape == (B, T, C), x.shape
    nc = _get_program()
    in_maps = _shard_inputs(x)
    res = bass_utils.run_bass_kernel_spmd(
        nc, in_maps, core_ids=list(range(N_CORES)), trace=trace
    )
    y = np.empty((B, T, C), np.float32)
    for k in range(N_CORES):
        yl = res.results[k]["y"].reshape(BG, C, T)
        y[BG * k : BG * (k + 1)] = yl.transpose(0, 2, 1).astype(np.float32)
    return y, res


def kernel(x):
    y, _ = run(x, trace=False)
    return y


# revision 16
# speedup vs baseline: 1.3489x; 1.3489x over previous
"""Trainium2 Bass kernel for a first-order IIR low-pass filter (v5b).

y_t = alpha * x_t + (1 - alpha) * y_{t-1},  y_{-1} = 0
x: [16, 65536, 64] float32  ->  y: [16, 65536, 64] float32

Structure: per tile of PT = 128*SD timesteps, partition p owns an
SD-step block.  PE matmuls against stationary beta-power matrices
compute each block's incoming state from a 128-step halo + the tile's
own (alpha-prescaled, bf16) inputs; the DVE then runs the SD-step
scan per partition (y_s = beta*y_{s-1} + x'_s), writing bf16.

v5b over v2b:
  - SD=32 (4 KB DMA runs both directions)
  - bf16 output wire (host upcasts), input already bf16
  - weights shipped pre-permuted -> linear load
  - all tile halos prepacked by the host into one [128, NT, BG, C]
    array, loaded in a single linear DMA upfront
  - output DMA split into two s-halves to drain earlier
"""

import math
import os
import sys

import numpy as np

try:
    import concourse.bass as bass
except ImportError:
    sys.path.insert(0, "/opt/trn_rl_repo")
    import concourse.bass as bass

import concourse.bacc as bacc
import concourse.mybir as mybir
import concourse.tile as tile
import ml_dtypes
from concourse import bass_utils

SAMPLE_RATE = 16000
CUTOFF_FREQ = 1000.0
_DT = 1.0 / SAMPLE_RATE
_TAU = 1.0 / (2.0 * math.pi * CUTOFF_FREQ)
ALPHA = _DT / (_DT + _TAU)
BETA = 1.0 - ALPHA

B, T, C = 16, 65536, 64
N_CORES = 8
BG = 4                    # batches per core
TH = T // 2               # timesteps per core
SD = int(os.environ.get("IIR_SD", "32"))   # timesteps per partition block
PT = 128 * SD             # timesteps per tile
NT = TH // PT             # tiles per core
HALO = 128                # history window feeding the state matmul

DT_IN = mybir.dt.bfloat16
XBUFS = int(os.environ.get("IIR_XBUFS", "4"))
YBUFS = int(os.environ.get("IIR_YBUFS", "4"))
PSBUFS = int(os.environ.get("IIR_PSBUFS", "6"))
YSPLIT = int(os.environ.get("IIR_YSPLIT", "2"))
# output dtype: bf16 on the wire (host upcasts); IIR_YF32=1 restores f32
Y_F32 = os.environ.get("IIR_YF32", "0") == "1"
DT_OUT = mybir.dt.float32 if Y_F32 else mybir.dt.bfloat16

_cached_nc = None


def _w_matrices():
    """Stationary weights for the state matmuls (lhsT layout [k, m]).

    W_s[k, m] = beta^(SD*(m-k) - 1 - s)  for m > k else 0
    H[k, m]  = beta^(127 - k + SD*m)
    (the scan runs on alpha-prescaled inputs, so no alpha here)
    """
    k = np.arange(128, dtype=np.float64)[:, None]
    m = np.arange(128, dtype=np.float64)[None, :]
    ws = np.zeros((SD, 128, 128), np.float64)
    for s in range(SD):
        e = SD * (m - k) - 1 - s
        ws[s] = np.where(e >= 0, BETA ** np.maximum(e, 0.0), 0.0)
    h = BETA ** (127.0 - k + SD * m)
    ws[np.abs(ws) < 1e-30] = 0.0
    h[np.abs(h) < 1e-30] = 0.0
    return ws.astype(ml_dtypes.bfloat16), h.astype(ml_dtypes.bfloat16)


def _build_program():
    nc = bacc.Bacc("TRN2", target_bir_lowering=False, debug=False)

    x_in = nc.dram_tensor("x", [BG, TH, C], DT_IN, kind="ExternalInput").ap()
    # host ships a_w pre-permuted to [k, s, m] so this load is linear
    a_w = nc.dram_tensor("a_w", [128, SD, 128], DT_IN, kind="ExternalInput").ap()
    a_h = nc.dram_tensor("a_h", [128, 128], DT_IN, kind="ExternalInput").ap()
    # all tile halos, prepacked [k, tile, b, c] -> one linear DMA
    a_halo = nc.dram_tensor(
        "a_halo", [HALO, NT, BG, C], DT_IN, kind="ExternalInput"
    ).ap()
    y_out = nc.dram_tensor("y", [BG, TH, C], DT_OUT, kind="ExternalOutput").ap()

    mult = mybir.AluOpType.mult
    add = mybir.AluOpType.add

    with tile.TileContext(nc) as tc:
        with (
            tc.tile_pool(name="w", bufs=1) as wpool,
            tc.tile_pool(name="xin", bufs=XBUFS) as xpool,
            tc.tile_pool(name="ysc", bufs=YBUFS) as ypool,
            tc.tile_pool(name="ps", bufs=PSBUFS, space="PSUM") as pspool,
        ):
            wt = wpool.tile([128, SD, 128], DT_IN, tag="wt")
            nc.sync.dma_start(wt[:], a_w[:])
            hw = wpool.tile([128, 128], DT_IN, tag="hw")
            nc.sync.dma_start(hw[:], a_h[:])
            halos = wpool.tile([HALO, NT, BG, C], DT_IN, tag="halos")
            nc.sync.dma_start(halos[:], a_halo[:])

            for j in range(NT):
                # [partition = SD-step block, batch, step, channel]; per
                # (p, b) the (step, channel) run is contiguous in DRAM.
                xt = xpool.tile([128, BG, SD, C], DT_IN, tag="xt")
                src = x_in[:, j * PT : (j + 1) * PT, :].rearrange(
                    "b (p s) c -> p b s c", p=128
                )
                nc.sync.dma_start(xt[:], src)

                # state entering each partition's window
                ps = pspool.tile([128, BG, C], mybir.dt.float32, tag="ps")
                nc.tensor.matmul(ps[:], hw[:], halos[:, j, :, :], start=True, stop=False)
                for s in range(SD):
                    nc.tensor.matmul(
                        ps[:], wt[:, s, :], xt[:, :, s, :],
                        start=False, stop=(s == SD - 1),
                    )

                # local SD-step scan per partition; inputs are
                # alpha-prescaled so this directly produces y
                yt = ypool.tile([128, BG, SD, C], DT_OUT, tag="yt")
                nc.vector.scalar_tensor_tensor(
                    yt[:, :, 0, :], ps[:], BETA, xt[:, :, 0, :], mult, add
                )
                for s in range(1, SD):
                    nc.vector.scalar_tensor_tensor(
                        yt[:, :, s, :], yt[:, :, s - 1, :], BETA,
                        xt[:, :, s, :], mult, add,
                    )

                sh = SD // YSPLIT
                dst = y_out[:, j * PT : (j + 1) * PT, :].rearrange(
                    "b (p s) c -> p b s c", p=128
                )
                for u in range(YSPLIT):
                    nc.scalar.dma_start(
                        dst[:, :, u * sh : (u + 1) * sh, :],
                        yt[:, :, u * sh : (u + 1) * sh, :],
                    )

    nc.compile()
    return nc


def _get_program():
    global _cached_nc
    if _cached_nc is None:
        _cached_nc = _build_program()
    return _cached_nc


def _shard_inputs(x):
    ws, h = _w_matrices()
    consts = {"a_w": np.ascontiguousarray(ws.transpose(1, 0, 2)), "a_h": h}
    xs = (np.float32(ALPHA) * x).astype(ml_dtypes.bfloat16)
    in_maps = []
    for g in range(4):
        for hh in range(2):
            b0 = BG * g
            t0 = TH * hh
            xl = np.ascontiguousarray(xs[b0 : b0 + BG, t0 : t0 + TH])
            # halo[k, j, b, c] = x'[b, t0 + j*PT - HALO + k, c] (0 for t<0)
            halo = np.zeros((HALO, NT, BG, C), ml_dtypes.bfloat16)
            for j in range(NT):
                ts = t0 + j * PT - HALO
                if ts >= 0:
                    halo[:, j] = xs[b0 : b0 + BG, ts : ts + HALO].transpose(1, 0, 2)
            in_maps.append({"x": xl, "a_halo": halo, **consts})
    return in_maps


def run(x, trace=False):
    x = np.ascontiguousarray(np.asarray(x, dtype=np.float32))
    assert x.shape == (B, T, C), x.shape
    nc = _get_program()
    in_maps = _shard_inputs(x)
    res = bass_utils.run_bass_kernel_spmd(
        nc, in_maps, core_ids=list(range(N_CORES)), trace=trace
    )
    y = np.empty((B, T, C), np.float32)
    core = 0
    for g in range(4):
        for hh in range(2):
            y[BG * g : BG * (g + 1), TH * hh : TH * (hh + 1)] = res.results[core][
                "y"
            ].astype(np.float32)
            core += 1
    return y, res


def kernel(x):
    y, _ = run(x, trace=False)
    return y


# revision 18
# speedup vs baseline: 1.4745x; 1.0931x over previous
"""Trainium2 Bass kernel for a first-order IIR low-pass filter (v7).

y_t = alpha * x_t + (1 - alpha) * y_{t-1},  y_{-1} = 0

All-matmul design: partition = step-in-window.  For each 128-step
window w,  Y_w = L^T X_w + H^T X_{w-1}  where
    L[k, m] = beta^(m-k)   (m >= k, else 0)     in-window prefix
    H[k, m] = beta^(m+128-k)                    halo (previous window)
computed by the PE into PSUM (f32), then cast-copied to SBUF bf16 by
DVE/ACT and DMA'd out.  beta^128 ~ 4e-19 so the halo is exact.

Sharding (8 cores): core k owns t in [k*8192, (k+1)*8192) for ALL
batches -> n = 16*64 = 1024 sequence columns.  The host packs
x[k, w, n] (step-major), so every DMA is linear with 16 KB
per-partition runs; the host unpacks y the same way.
"""

import math
import os
import sys

import numpy as np

try:
    import concourse.bass as bass
except ImportError:
    sys.path.insert(0, "/opt/trn_rl_repo")
    import concourse.bass as bass

import concourse.bacc as bacc
import concourse.mybir as mybir
import concourse.tile as tile
import ml_dtypes
from concourse import bass_utils

SAMPLE_RATE = 16000
CUTOFF_FREQ = 1000.0
_DT = 1.0 / SAMPLE_RATE
_TAU = 1.0 / (2.0 * math.pi * CUTOFF_FREQ)
ALPHA = _DT / (_DT + _TAU)
BETA = 1.0 - ALPHA

B, T, C = 16, 65536, 64
N_CORES = 8
N = B * C                   # sequence columns per core (1024)
TS = T // N_CORES           # timesteps per core (8192)
NW = TS // 128              # 128-step windows per core (64)
WT = int(os.environ.get("IIR_WT", "8"))    # windows per tile
NT = NW // WT               # tiles per core
NH = 512                    # matmul free-dim half (PSUM f32 bank limit)

DT_IN = mybir.dt.bfloat16
XBUFS = int(os.environ.get("IIR_XBUFS", "3"))
YBUFS = int(os.environ.get("IIR_YBUFS", "3"))
PSBUFS = int(os.environ.get("IIR_PSBUFS", "6"))
# fraction of PSUM->SBUF copies on ACT: every ACT_EVERY-th window-half
ACT_EVERY = int(os.environ.get("IIR_ACT_EVERY", "3"))

_cached_nc = None


def _lh_matrices():
    k = np.arange(128, dtype=np.float64)[:, None]
    m = np.arange(128, dtype=np.float64)[None, :]
    l = np.where(m >= k, BETA ** np.maximum(m - k, 0.0), 0.0)
    h = BETA ** (m + 128.0 - k)
    l[np.abs(l) < 1e-30] = 0.0
    h[np.abs(h) < 1e-30] = 0.0
    return l.astype(ml_dtypes.bfloat16), h.astype(ml_dtypes.bfloat16)


def _build_program():
    nc = bacc.Bacc("TRN2", target_bir_lowering=False, debug=False)

    x_in = nc.dram_tensor("x", [128, NW, N], DT_IN, kind="ExternalInput").ap()
    a_l = nc.dram_tensor("a_l", [128, 128], DT_IN, kind="ExternalInput").ap()
    a_h = nc.dram_tensor("a_h", [128, 128], DT_IN, kind="ExternalInput").ap()
    a_halo = nc.dram_tensor("a_halo", [128, N], DT_IN, kind="ExternalInput").ap()
    y_out = nc.dram_tensor("y", [128, NW, N], DT_IN, kind="ExternalOutput").ap()

    with tile.TileContext(nc) as tc:
        with (
            tc.tile_pool(name="w", bufs=1) as wpool,
            tc.tile_pool(name="xin", bufs=XBUFS) as xpool,
            tc.tile_pool(name="yst", bufs=YBUFS) as ypool,
            tc.tile_pool(name="ps", bufs=PSBUFS, space="PSUM") as pspool,
        ):
            lw = wpool.tile([128, 128], DT_IN, tag="lw")
            nc.sync.dma_start(lw[:], a_l[:])
            hw = wpool.tile([128, 128], DT_IN, tag="hw")
            nc.sync.dma_start(hw[:], a_h[:])
            halo0 = wpool.tile([128, N], DT_IN, tag="halo0")
            nc.sync.dma_start(halo0[:], a_halo[:])

            prev_xt = None
            ci = 0  # copy counter for DVE/ACT split
            for j in range(NT):
                xt = xpool.tile([128, WT, N], DT_IN, tag="xt")
                nc.sync.dma_start(xt[:], x_in[:, j * WT : (j + 1) * WT, :])

                yt = ypool.tile([128, WT, N], DT_IN, tag="yt")
                for w in range(WT):
                    if j == 0 and w == 0:
                        xprev = halo0[:]
                    elif w == 0:
                        xprev = prev_xt[:, WT - 1, :]
                    else:
                        xprev = xt[:, w - 1, :]
                    for u in range(N // NH):
                        ps = pspool.tile([128, NH], mybir.dt.float32, tag="ps")
                        nc.tensor.matmul(
                            ps[:], hw[:], xprev[:, u * NH : (u + 1) * NH],
                            start=True, stop=False,
                        )
                        nc.tensor.matmul(
                            ps[:], lw[:], xt[:, w, u * NH : (u + 1) * NH],
                            start=False, stop=True,
                        )
                        dst = yt[:, w, u * NH : (u + 1) * NH]
                        if ACT_EVERY > 0 and ci % ACT_EVERY == ACT_EVERY - 1:
                            nc.scalar.activation(
                                dst, ps[:], mybir.ActivationFunctionType.Copy
                            )
                        else:
                            nc.vector.tensor_copy(dst, ps[:])
                        ci += 1
                prev_xt = xt

                nc.scalar.dma_start(y_out[:, j * WT : (j + 1) * WT, :], yt[:])

    nc.compile()
    return nc


def _get_program():
    global _cached_nc
    if _cached_nc is None:
        _cached_nc = _build_program()
    return _cached_nc


def _shard_inputs(x):
    l, h = _lh_matrices()
    xs = (np.float32(ALPHA) * x).astype(ml_dtypes.bfloat16)  # [B, T, C]
    in_maps = []
    for k in range(N_CORES):
        t0 = k * TS
        slab = xs[:, t0 : t0 + TS, :]                    # [B, TS, C]
        xl = np.ascontiguousarray(
            slab.reshape(B, NW, 128, C).transpose(2, 1, 0, 3)
        ).reshape(128, NW, N)
        halo = np.zeros((128, N), ml_dtypes.bfloat16)
        if k > 0:
            halo[:] = (
                xs[:, t0 - 128 : t0, :].transpose(1, 0, 2).reshape(128, N)
            )
        in_maps.append({"x": xl, "a_halo": halo, "a_l": l, "a_h": h})
    return in_maps


def run(x, trace=False):
    x = np.ascontiguousarray(np.asarray(x, dtype=np.float32))
    assert x.shape == (B, T, C), x.shape
    nc = _get_program()
    in_maps = _shard_inputs(x)
    res = bass_utils.run_bass_kernel_spmd(
        nc, in_maps, core_ids=list(range(N_CORES)), trace=trace
    )
    y = np.empty((B, T, C), np.float32)
    for k in range(N_CORES):
        t0 = k * TS
        yl = res.results[k]["y"].reshape(128, NW, B, C)
        y[:, t0 : t0 + TS, :] = (
            yl.transpose(2, 1, 0, 3).reshape(B, TS, C).astype(np.float32)
        )
    return y, res


def kernel(x):
    y, _ = run(x, trace=False)
    return y
